# revision 17
# baseline (speedup 1.0000x reference)
"""BRD4KANModel Trainium2 kernel (v2).

Data-parallel over batch across 8 NeuronCores (512 rows each, weights
replicated). All weights are pre-transposed / pre-tiled / bf16-cast on the
host into matmul-ready lhsT layout ([in-feature partitions, out-feature
free]), with the spline scaler and the truncated-power scale lam folded in.
The device therefore runs ONLY real matmuls on the PE (no transposes), the
B-spline bases on ACT+DVE(+POOL), and PSUM evacuations.

B-spline bases via truncated powers: with h' = lam*h and c_m = lam*g_m,
z_m = relu(h' - c_m), the 6 cubic bases are the 4th forward differences
b_c = z³_c - 4z³_{c+1} + 6z³_{c+2} - 4z³_{c+3} + z³_{c+4}, computed as a
grouped 24-op DVE cascade per 128-feature tile. z² comes from one ACT
Square (bias = -c_m) and z³ = relu * z² runs on POOL (or DVE via env
BASS_CUBE=dve).

Layer matmuls are split into 4 k-quarters (i-tiles 0-3, 4-7, 8-11, 12-15)
so only ~2 quarters of bases tiles are ever live (SBUF), with an f32 SBUF
accumulator carrying partial sums between quarters. Bases for the next
consumer sweep are emitted one sweep ahead, overlapping DVE/ACT/POOL work
with PE matmuls.

This walrus build accepts only ONE semaphore wait per instruction, while
Tile's scheduler attaches several; _split_waits() post-processes the BIR
JSON, hoisting excess waits onto NoOps inserted just before each
instruction on the same engine.
"""

import json
import os

import numpy as np
import ml_dtypes

import concourse.bass as bass
import concourse.mybir as mybir
import concourse.tile as tile

F32 = mybir.dt.float32
BF16 = mybir.dt.bfloat16
AF = mybir.ActivationFunctionType
OP = mybir.AluOpType

N_CORES = 8
BATCH = 4096
B = BATCH // N_CORES  # 512 per core
D = 2048
WIDTHS = [2048, 2048, 1024]
COEFF = 6
GRID_SIZE = 3
SPLINE_ORDER = 3
HSTEP = 2.0 / GRID_SIZE
GRID = [m * HSTEP - 1.0 - SPLINE_ORDER * HSTEP
        for m in range(GRID_SIZE + 2 * SPLINE_ORDER + 1)]  # 10 knots, -3..3
LAM = float((6.0 * HSTEP ** 3) ** (-1.0 / 3.0))
NK = 10          # truncated-power knots
IT = 16          # 2048/128 input tiles per layer
NQ = 4           # k-quarters
KQ = IT // NQ    # i-tiles per quarter
SW = 512 + KQ * COEFF * 128  # combined strip width per (o,q): base + spline


def _split_waits(bir_bytes: bytes, keep: int = 1) -> bytes:
    d = json.loads(bir_bytes)
    for f in d["functions"]:
        for bb in f["blocks"]:
            new_insts = []
            for inst in bb["instructions"]:
                si = inst.get("sync_info")
                waits = (si or {}).get("on_wait") or []
                if len(waits) > keep:
                    extra = waits[:-keep]
                    inst["sync_info"]["on_wait"] = waits[-keep:]
                    for ci in range(0, len(extra), keep):
                        new_insts.append({
                            "name": f"{inst['name']}-w{ci}",
                            "opcode": "NoOp",
                            "engine": inst["engine"],
                            "ins": [],
                            "outs": [],
                            "debug": inst.get("debug"),
                            "sync_info": {"on_update": [],
                                          "on_wait": extra[ci:ci + keep]},
                        })
                new_insts.append(inst)
            bb["instructions"] = new_insts
    return json.dumps(d).encode()


def _patch_json(nc):
    orig = nc.to_json_bytes

    def patched():
        return _split_waits(orig())

    nc.to_json_bytes = patched
    return nc


def build():
    nc = bass.Bass()
    xT = nc.dram_tensor("xT", [D, B], BF16, kind="ExternalInput")
    wm = nc.dram_tensor("wm", [32 * 128, D], BF16, kind="ExternalInput")
    mbg = nc.dram_tensor("mbg", [128, 16], F32, kind="ExternalInput")
    mbv = nc.dram_tensor("mbv", [128, 16], F32, kind="ExternalInput")
    ws_d = []
    for l, fo in enumerate(WIDTHS):
        ot = fo // 128
        ws_d.append(nc.dram_tensor(f"ws{l}", [ot * NQ * 128, SW], BF16,
                                   kind="ExternalInput"))
    wh = nc.dram_tensor("wh", [128, 16], BF16, kind="ExternalInput")
    hb = nc.dram_tensor("hb", [2, 1], F32, kind="ExternalInput")
    out = nc.dram_tensor("out", [2, B], F32, kind="ExternalOutput")

    with tile.TileContext(nc) as tc:
        with tc.tile_pool(name="consts", bufs=1) as consts, \
             tc.tile_pool(name="wmp", bufs=2) as wmp, \
             tc.tile_pool(name="wsp", bufs=2) as wsp, \
             tc.tile_pool(name="hp", bufs=34) as hp, \
             tc.tile_pool(name="silup", bufs=21) as silup, \
             tc.tile_pool(name="basp", bufs=49) as basp, \
             tc.tile_pool(name="zp", bufs=10) as zp, \
             tc.tile_pool(name="rtp", bufs=6) as rtp, \
             tc.tile_pool(name="qp", bufs=6) as qp, \
             tc.tile_pool(name="psA", bufs=6, space="PSUM") as psA, \
             tc.tile_pool(name="psH", bufs=1, space="PSUM") as psH:

            # ---- constants ----
            cm = consts.tile([128, NK], F32, tag="cm")
            for m in range(NK):
                nc.vector.memset(cm[:, m:m + 1], float(-LAM * GRID[m]))
            mbg_sb = consts.tile([128, 16], F32, tag="mbg")
            nc.sync.dma_start(mbg_sb, mbg[:])
            mbv_sb = consts.tile([128, 16], F32, tag="mbv")
            nc.sync.dma_start(mbv_sb, mbv[:])
            wh_sb = consts.tile([128, 16], BF16, tag="wh")
            nc.sync.dma_start(wh_sb, wh[:])
            hb_sb = consts.tile([2, 1], F32, tag="hb")
            nc.sync.dma_start(hb_sb, hb[:])

            # ---- x^T tiles (host pre-transposed; share bas slots) ----
            xb = []
            for i in range(IT):
                t = basp.tile([128, B], BF16, tag="bas", name=f"x{i}")
                nc.sync.dma_start(t, xT[i * 128:(i + 1) * 128, :])
                xb.append(t)

            silu_t = {}
            bas_t = {}

            def emit_silu(l, i, h_t):
                st = silup.tile([128, B], BF16, tag="silu",
                                name=f"silu{l}_{i}")
                nc.scalar.activation(st, h_t, AF.Silu, scale=1.0 / LAM)
                silu_t[(l, i)] = st

            def emit_A(l, i, h_t, na):
                """6 b-spline bases tiles for layer-l input tile i.

                na of the 10 z^3 cubes go through ACT (exp(3*ln(relu))),
                the rest use ACT relu+square plus a DVE multiply. Splitting
                balances the two engines (DVE and ACT+POOL share nothing;
                the GPSIMD SBUF port contention makes POOL offload a wash).
                """
                z3 = []
                for m in range(NK):
                    r = rtp.tile([128, B], F32, tag="rt", name=f"r{m}")
                    nc.scalar.activation(r, h_t, AF.Relu,
                                         bias=cm[:, m:m + 1])
                    z = zp.tile([128, B], F32, tag="z", name=f"z3_{m}")
                    if m < na:
                        lnr = rtp.tile([128, B], F32, tag="rt",
                                       name=f"lnr{m}")
                        nc.scalar.activation(lnr, r, AF.Ln)
                        nc.scalar.activation(z, lnr, AF.Exp, scale=3.0)
                    else:
                        t2 = rtp.tile([128, B], F32, tag="rt",
                                      name=f"t2{m}")
                        nc.scalar.activation(t2, h_t, AF.Square,
                                             bias=cm[:, m:m + 1])
                        nc.vector.tensor_tensor(z, r, t2, OP.mult)
                    z3.append(z)
                for c in range(COEFF):
                    q = qp.tile([128, B], F32, tag="q", name=f"q{c}")
                    nc.vector.tensor_tensor(q, z3[c], z3[c + 4], OP.add)
                    r2 = qp.tile([128, B], F32, tag="q", name=f"r2{c}")
                    nc.vector.tensor_tensor(r2, z3[c + 1], z3[c + 3], OP.add)
                    nc.vector.scalar_tensor_tensor(q, r2, -4.0, q,
                                                   OP.mult, OP.add)
                    bt = basp.tile([128, B], BF16, tag="bas",
                                   name=f"bas{l}_{i}_{c}")
                    nc.vector.scalar_tensor_tensor(bt, z3[c + 2], 6.0, q,
                                                   OP.mult, OP.add)
                    bas_t[(l, i, c)] = bt

            # ---- multiplicative layer ----
            h_cur = []
            for j in range(IT):
                wg = wmp.tile([128, D], BF16, tag="wm", name=f"wg{j}")
                nc.sync.dma_start(wg, wm[j * 128:(j + 1) * 128, :])
                accg = psA.tile([128, B], F32, tag="acc")
                for k in range(IT):
                    nc.tensor.matmul(accg, wg[:, k * 128:(k + 1) * 128],
                                     xb[k], start=(k == 0),
                                     stop=(k == IT - 1))
                sig = qp.tile([128, B], F32, tag="q", name=f"sig{j}")
                nc.scalar.activation(sig, accg, AF.Sigmoid,
                                     bias=mbg_sb[:, j:j + 1])
                wv = wmp.tile([128, D], BF16, tag="wm", name=f"wv{j}")
                nc.sync.dma_start(wv, wm[(16 + j) * 128:(17 + j) * 128, :])
                accv = psA.tile([128, B], F32, tag="acc")
                for k in range(IT):
                    nc.tensor.matmul(accv, wv[:, k * 128:(k + 1) * 128],
                                     xb[k], start=(k == 0),
                                     stop=(k == IT - 1))
                ht = hp.tile([128, B], F32, tag="h", name=f"h0_{j}")
                nc.vector.scalar_tensor_tensor(ht, accv, mbv_sb[:, j:j + 1],
                                               sig, OP.add, OP.mult)
                h_cur.append(ht)
                # bases for quarter 0 only: more would exhaust bas slots
                # (shared with x tiles) and wedge the DVE queue behind slot
                # waits. DVE cubes (na=0): no ln/exp while the sigmoid
                # table-set is live.
                if j < KQ:
                    emit_A(0, j, ht, 0)
            # batched so the ACT table-set switches once, not per tile
            for j in range(IT):
                emit_silu(0, j, h_cur[j])

            # ---- KAN layers: 4-quarter k-split sweeps ----
            h3 = []
            for l in range(3):
                ot = WIDTHS[l] // 128
                hacc = [None] * ot
                for q in range(NQ):
                    for o in range(ot):
                        strip = wsp.tile([128, SW], BF16, tag="ws",
                                         name=f"ws{l}_{q}_{o}")
                        row = (o * NQ + q) * 128
                        nc.sync.dma_start(strip, ws_d[l][row:row + 128, :])
                        acc = psA.tile([128, B], F32, tag="acc")
                        idx = 0
                        last = KQ * (1 + COEFF) - 1
                        for kk in range(KQ):
                            i = q * KQ + kk
                            nc.tensor.matmul(
                                acc, strip[:, kk * 128:(kk + 1) * 128],
                                silu_t[(l, i)], start=(idx == 0),
                                stop=(idx == last))
                            idx += 1
                            for c in range(COEFF):
                                o0 = 512 + (kk * COEFF + c) * 128
                                nc.tensor.matmul(
                                    acc, strip[:, o0:o0 + 128],
                                    bas_t[(l, i, c)], start=False,
                                    stop=(idx == last))
                                idx += 1
                        if q == 0:
                            hacc[o] = hp.tile([128, B], F32, tag="h",
                                              name=f"hacc{l}_{o}")
                            nc.scalar.copy(hacc[o], acc)
                        elif q < NQ - 1 or l < 2:
                            nc.vector.tensor_tensor(hacc[o], acc, hacc[o],
                                                    OP.add)
                        else:
                            h3t = silup.tile([128, B], BF16, tag="silu",
                                             name=f"h3_{o}")
                            nc.vector.tensor_tensor(h3t, acc, hacc[o],
                                                    OP.add)
                            h3.append(h3t)
                        # weave bases one quarter ahead: during B(l,q) build
                        # bases for quarter q+1 (slots freed by sweep q-1).
                        # At q3, batch next layer's silus (table-set
                        # grouping) and its quarter-0 bases.
                        if l + 1 < 3 and q == NQ - 1:
                            emit_silu(l + 1, o, hacc[o])
                            if o == KQ - 1:
                                # na=0: relu/square only, so the silu-set
                                # stays loaded across this boundary window
                                for oo in range(KQ):
                                    emit_A(l + 1, oo, hacc[oo], 0)
                        if q < NQ - 1 and o < KQ:
                            emit_A(l, KQ * (q + 1) + o,
                                   h_cur[KQ * (q + 1) + o],
                                   10 if l == 2 else 5)
                h_cur = hacc

            # ---- heads ----
            acch = psH.tile([128, B], F32, tag="acch")
            it2 = WIDTHS[-1] // 128
            for k in range(it2):
                nc.tensor.matmul(acch[0:2, :], wh_sb[:, 2 * k:2 * k + 2],
                                 h3[k], start=(k == 0), stop=(k == it2 - 1))
            res = consts.tile([2, B], F32, tag="res")
            nc.vector.tensor_scalar(res, acch[0:2, :], hb_sb[:, 0:1], None,
                                    OP.add)
            nc.sync.dma_start(out[:], res)

    return _patch_json(nc)


def _prep(inputs):
    """Host-side weight prep: fold scaler+lam, transpose, tile, bf16-cast."""
    f32 = np.float32
    bf16 = ml_dtypes.bfloat16
    feed = {}

    mw = np.asarray(inputs["mult_w"], f32).copy()  # [4096, 2048]
    mw[D:] *= LAM
    feed["wm"] = np.ascontiguousarray(
        mw.reshape(32, 128, IT, 128).transpose(0, 3, 2, 1)
        .reshape(32 * 128, D)).astype(bf16)
    mb = np.asarray(inputs["mult_b"], f32)
    feed["mbg"] = np.ascontiguousarray(mb[:D].reshape(16, 128).T).astype(f32)
    feed["mbv"] = np.ascontiguousarray(
        (LAM * mb[D:]).reshape(16, 128).T).astype(f32)

    for l, fo in enumerate(WIDTHS):
        sc_out = LAM if l < 2 else 1.0
        bw = np.asarray(inputs[f"base_w{l}"], f32) * sc_out
        sw = (np.asarray(inputs[f"spline_w{l}"], f32)
              * np.asarray(inputs[f"scaler{l}"], f32)[..., None] * sc_out)
        ot = fo // 128
        bwt = bw.reshape(ot, 128, IT, 128).transpose(0, 3, 2, 1)
        swt = sw.reshape(ot, 128, IT, 128, COEFF).transpose(0, 3, 2, 4, 1)
        arr = np.empty((ot, NQ, 128, SW), f32)
        arr[:, :, :, :512] = (bwt.reshape(ot, 128, NQ, KQ, 128)
                              .transpose(0, 2, 1, 3, 4)
                              .reshape(ot, NQ, 128, KQ * 128))
        arr[:, :, :, 512:] = (swt.reshape(ot, 128, NQ, KQ, COEFF, 128)
                              .transpose(0, 2, 1, 3, 4, 5)
                              .reshape(ot, NQ, 128, KQ * COEFF * 128))
        feed[f"ws{l}"] = np.ascontiguousarray(
            arr.reshape(ot * NQ * 128, SW)).astype(bf16)

    whh = np.stack([np.asarray(inputs["reg_w"], f32)[0],
                    np.asarray(inputs["aux_w"], f32)[0]], axis=1)  # [1024,2]
    feed["wh"] = np.ascontiguousarray(
        whh.reshape(8, 128, 2).transpose(1, 0, 2).reshape(128, 16)
    ).astype(bf16)
    feed["hb"] = np.array([[np.asarray(inputs["reg_b"], f32)[0]],
                           [np.asarray(inputs["aux_b"], f32)[0]]], f32)
    return feed


_NC = None


def kernel(**inputs):
    global _NC
    from concourse.bass_utils import run_bass_kernel_spmd

    if _NC is None:
        _NC = build()
    shared = _prep(inputs)
    x_full = np.asarray(inputs["x"], np.float32)
    per_core = []
    for c in range(N_CORES):
        m = dict(shared)
        m["xT"] = np.ascontiguousarray(
            x_full[c * B:(c + 1) * B].T).astype(ml_dtypes.bfloat16)
        per_core.append(m)
    res = run_bass_kernel_spmd(_NC, per_core, core_ids=list(range(N_CORES)))
    reg = np.concatenate([res.results[c]["out"][0] for c in range(N_CORES)])
    aux = np.concatenate([res.results[c]["out"][1] for c in range(N_CORES)])
    kernel.last_results = res
    return reg, aux


# revision 21
# speedup vs baseline: 1.1344x; 1.1344x over previous
"""BRD4KANModel Trainium2 kernel (v2).

Data-parallel over batch across 8 NeuronCores (512 rows each, weights
replicated). All weights are pre-transposed / pre-tiled / bf16-cast on the
host into matmul-ready lhsT layout ([in-feature partitions, out-feature
free]), with the spline scaler and the truncated-power scale lam folded in.
The device therefore runs ONLY real matmuls on the PE (no transposes), the
B-spline bases on ACT+DVE(+POOL), and PSUM evacuations.

B-spline bases via truncated powers: with h' = lam*h and c_m = lam*g_m,
z_m = relu(h' - c_m), the 6 cubic bases are the 4th forward differences
b_c = z³_c - 4z³_{c+1} + 6z³_{c+2} - 4z³_{c+3} + z³_{c+4}, computed as a
grouped 24-op DVE cascade per 128-feature tile. z² comes from one ACT
Square (bias = -c_m) and z³ = relu * z² runs on POOL (or DVE via env
BASS_CUBE=dve).

Layer matmuls are split into 4 k-quarters (i-tiles 0-3, 4-7, 8-11, 12-15)
so only ~2 quarters of bases tiles are ever live (SBUF), with an f32 SBUF
accumulator carrying partial sums between quarters. Bases for the next
consumer sweep are emitted one sweep ahead, overlapping DVE/ACT/POOL work
with PE matmuls.

This walrus build accepts only ONE semaphore wait per instruction, while
Tile's scheduler attaches several; _split_waits() post-processes the BIR
JSON, hoisting excess waits onto NoOps inserted just before each
instruction on the same engine.
"""

import json
import os

import numpy as np
import ml_dtypes

import concourse.bass as bass
import concourse.mybir as mybir
import concourse.tile as tile

F32 = mybir.dt.float32
BF16 = mybir.dt.bfloat16
AF = mybir.ActivationFunctionType
OP = mybir.AluOpType

N_CORES = 8
BATCH = 4096
B = BATCH // N_CORES  # 512 per core
D = 2048
WIDTHS = [2048, 2048, 1024]
COEFF = 6
GRID_SIZE = 3
SPLINE_ORDER = 3
HSTEP = 2.0 / GRID_SIZE
GRID = [m * HSTEP - 1.0 - SPLINE_ORDER * HSTEP
        for m in range(GRID_SIZE + 2 * SPLINE_ORDER + 1)]  # 10 knots, -3..3
LAM = float((6.0 * HSTEP ** 3) ** (-1.0 / 3.0))
NK = 10          # truncated-power knots
IT = 16          # 2048/128 input tiles per layer
NQ = 4           # k-quarters
KQ = IT // NQ    # i-tiles per quarter
SW = 512 + KQ * COEFF * 128  # combined strip width per (o,q): base + spline


def _split_waits(bir_bytes: bytes, keep: int = 1) -> bytes:
    d = json.loads(bir_bytes)
    for f in d["functions"]:
        for bb in f["blocks"]:
            new_insts = []
            for inst in bb["instructions"]:
                si = inst.get("sync_info")
                waits = (si or {}).get("on_wait") or []
                if len(waits) > keep:
                    extra = waits[:-keep]
                    inst["sync_info"]["on_wait"] = waits[-keep:]
                    for ci in range(0, len(extra), keep):
                        new_insts.append({
                            "name": f"{inst['name']}-w{ci}",
                            "opcode": "NoOp",
                            "engine": inst["engine"],
                            "ins": [],
                            "outs": [],
                            "debug": inst.get("debug"),
                            "sync_info": {"on_update": [],
                                          "on_wait": extra[ci:ci + keep]},
                        })
                new_insts.append(inst)
            bb["instructions"] = new_insts
    return json.dumps(d).encode()


def _patch_json(nc):
    orig = nc.to_json_bytes

    def patched():
        return _split_waits(orig())

    nc.to_json_bytes = patched
    return nc


def build():
    nc = bass.Bass()
    xT = nc.dram_tensor("xT", [D, B], BF16, kind="ExternalInput")
    wm = nc.dram_tensor("wm", [32 * 128, D], BF16, kind="ExternalInput")
    mbg = nc.dram_tensor("mbg", [128, 16], F32, kind="ExternalInput")
    mbv = nc.dram_tensor("mbv", [128, 16], F32, kind="ExternalInput")
    ws_d = []
    for l, fo in enumerate(WIDTHS):
        ot = fo // 128
        ws_d.append(nc.dram_tensor(f"ws{l}", [ot * NQ * 128, SW], BF16,
                                   kind="ExternalInput"))
    wh = nc.dram_tensor("wh", [128, 16], BF16, kind="ExternalInput")
    hb = nc.dram_tensor("hb", [2, 1], F32, kind="ExternalInput")
    out = nc.dram_tensor("out", [2, B], F32, kind="ExternalOutput")

    with tile.TileContext(nc) as tc:
        with tc.tile_pool(name="consts", bufs=1) as consts, \
             tc.tile_pool(name="wmp", bufs=2) as wmp, \
             tc.tile_pool(name="wsp", bufs=2) as wsp, \
             tc.tile_pool(name="hp", bufs=34) as hp, \
             tc.tile_pool(name="silup", bufs=21) as silup, \
             tc.tile_pool(name="basp", bufs=49) as basp, \
             tc.tile_pool(name="zp", bufs=10) as zp, \
             tc.tile_pool(name="rtp", bufs=6) as rtp, \
             tc.tile_pool(name="qp", bufs=6) as qp, \
             tc.tile_pool(name="psA", bufs=6, space="PSUM") as psA, \
             tc.tile_pool(name="psH", bufs=1, space="PSUM") as psH:

            # ---- constants ----
            cm = consts.tile([128, NK], F32, tag="cm")
            for m in range(NK):
                nc.vector.memset(cm[:, m:m + 1], float(-LAM * GRID[m]))
            mbg_sb = consts.tile([128, 16], F32, tag="mbg")
            nc.sync.dma_start(mbg_sb, mbg[:])
            mbv_sb = consts.tile([128, 16], F32, tag="mbv")
            nc.sync.dma_start(mbv_sb, mbv[:])
            wh_sb = consts.tile([128, 16], BF16, tag="wh")
            nc.sync.dma_start(wh_sb, wh[:])
            hb_sb = consts.tile([2, 1], F32, tag="hb")
            nc.sync.dma_start(hb_sb, hb[:])

            # ---- x^T tiles (host pre-transposed; share bas slots) ----
            xb = []
            for i in range(IT):
                t = basp.tile([128, B], BF16, tag="bas", name=f"x{i}")
                nc.sync.dma_start(t, xT[i * 128:(i + 1) * 128, :])
                xb.append(t)

            silu_t = {}
            bas_t = {}

            def emit_silu(l, i, h_t, bias=0.0):
                # bias is always zero-valued; passing a gate tile written by
                # a late DVE op delays readiness so the list scheduler can't
                # pull Silu (its own ACT table-set) into an earlier window
                st = silup.tile([128, B], BF16, tag="silu",
                                name=f"silu{l}_{i}")
                nc.scalar.activation(st, h_t, AF.Silu, scale=1.0 / LAM,
                                     bias=bias)
                silu_t[(l, i)] = st

            def emit_A(l, i, h_t, na):
                """6 b-spline bases tiles for layer-l input tile i.

                na of the 10 z^3 cubes go through ACT (exp(3*ln(relu))),
                the rest use ACT relu+square plus a DVE multiply. Splitting
                balances the two engines (DVE and ACT+POOL share nothing;
                the GPSIMD SBUF port contention makes POOL offload a wash).
                """
                z3 = []
                for m in range(NK):
                    r = rtp.tile([128, B], F32, tag="rt", name=f"r{m}")
                    nc.scalar.activation(r, h_t, AF.Relu,
                                         bias=cm[:, m:m + 1])
                    z = zp.tile([128, B], F32, tag="z", name=f"z3_{m}")
                    if m < na:
                        lnr = rtp.tile([128, B], F32, tag="rt",
                                       name=f"lnr{m}")
                        nc.scalar.activation(lnr, r, AF.Ln)
                        nc.scalar.activation(z, lnr, AF.Exp, scale=3.0)
                    else:
                        t2 = rtp.tile([128, B], F32, tag="rt",
                                      name=f"t2{m}")
                        nc.scalar.activation(t2, h_t, AF.Square,
                                             bias=cm[:, m:m + 1])
                        nc.vector.tensor_tensor(z, r, t2, OP.mult)
                    z3.append(z)
                for c in range(COEFF):
                    q = qp.tile([128, B], F32, tag="q", name=f"q{c}")
                    nc.vector.tensor_tensor(q, z3[c], z3[c + 4], OP.add)
                    r2 = qp.tile([128, B], F32, tag="q", name=f"r2{c}")
                    nc.vector.tensor_tensor(r2, z3[c + 1], z3[c + 3], OP.add)
                    nc.vector.scalar_tensor_tensor(q, r2, -4.0, q,
                                                   OP.mult, OP.add)
                    bt = basp.tile([128, B], BF16, tag="bas",
                                   name=f"bas{l}_{i}_{c}")
                    nc.vector.scalar_tensor_tensor(bt, z3[c + 2], 6.0, q,
                                                   OP.mult, OP.add)
                    bas_t[(l, i, c)] = bt

            # ---- multiplicative layer ----
            h_cur = []
            for j in range(IT):
                wg = wmp.tile([128, D], BF16, tag="wm", name=f"wg{j}")
                nc.sync.dma_start(wg, wm[j * 128:(j + 1) * 128, :])
                accg = psA.tile([128, B], F32, tag="acc")
                for k in range(IT):
                    nc.tensor.matmul(accg, wg[:, k * 128:(k + 1) * 128],
                                     xb[k], start=(k == 0),
                                     stop=(k == IT - 1))
                sig = qp.tile([128, B], F32, tag="q", name=f"sig{j}")
                nc.scalar.activation(sig, accg, AF.Sigmoid,
                                     bias=mbg_sb[:, j:j + 1])
                wv = wmp.tile([128, D], BF16, tag="wm", name=f"wv{j}")
                nc.sync.dma_start(wv, wm[(16 + j) * 128:(17 + j) * 128, :])
                accv = psA.tile([128, B], F32, tag="acc")
                for k in range(IT):
                    nc.tensor.matmul(accv, wv[:, k * 128:(k + 1) * 128],
                                     xb[k], start=(k == 0),
                                     stop=(k == IT - 1))
                ht = hp.tile([128, B], F32, tag="h", name=f"h0_{j}")
                nc.vector.scalar_tensor_tensor(ht, accv, mbv_sb[:, j:j + 1],
                                               sig, OP.add, OP.mult)
                h_cur.append(ht)
            # A(0,q0) only after the mult loop: its DVE cascades must not
            # sit ahead of the h0 evac STTs in the DVE stream (PSUM bank
            # starvation stalls the PE). na=0: no ln/exp while the sigmoid
            # table-set is live. Quarter 0 only: more would exhaust bas
            # slots (shared with x tiles).
            for j in range(KQ):
                emit_A(0, j, h_cur[j], 0)
            # zero tile written only after the last mult evac: gates the
            # silu batch out of the sigmoid-set window
            z00 = qp.tile([128, 1], F32, tag="zb", bufs=2)
            nc.vector.tensor_scalar(z00, h_cur[IT - 1][:, 0:1], 0.0, None,
                                    OP.mult)
            for j in range(IT):
                emit_silu(0, j, h_cur[j], bias=z00)

            # ---- KAN layers: 4-quarter k-split sweeps ----
            h3 = []
            for l in range(3):
                ot = WIDTHS[l] // 128
                hacc = [None] * ot
                for q in range(NQ):
                    zb = None
                    if l + 1 < 3 and q == NQ - 1:
                        # gate for next layer's silu batch: ready only after
                        # this layer's last bases tile (i.e. after all ln/exp
                        # ACT work for layer l is done)
                        zb = qp.tile([128, 1], F32, tag="zb", bufs=2,
                                     name=f"zb{l}")
                        nc.vector.tensor_scalar(
                            zb, bas_t[(l, IT - 1, COEFF - 1)][:, 0:1], 0.0,
                            None, OP.mult)
                    for o in range(ot):
                        strip = wsp.tile([128, SW], BF16, tag="ws",
                                         name=f"ws{l}_{q}_{o}")
                        row = (o * NQ + q) * 128
                        nc.sync.dma_start(strip, ws_d[l][row:row + 128, :])
                        acc = psA.tile([128, B], F32, tag="acc")
                        idx = 0
                        last = KQ * (1 + COEFF) - 1
                        for kk in range(KQ):
                            i = q * KQ + kk
                            nc.tensor.matmul(
                                acc, strip[:, kk * 128:(kk + 1) * 128],
                                silu_t[(l, i)], start=(idx == 0),
                                stop=(idx == last))
                            idx += 1
                            for c in range(COEFF):
                                o0 = 512 + (kk * COEFF + c) * 128
                                nc.tensor.matmul(
                                    acc, strip[:, o0:o0 + 128],
                                    bas_t[(l, i, c)], start=False,
                                    stop=(idx == last))
                                idx += 1
                        if q == 0:
                            hacc[o] = hp.tile([128, B], F32, tag="h",
                                              name=f"hacc{l}_{o}")
                            nc.scalar.copy(hacc[o], acc)
                        elif q < NQ - 1 or l < 2:
                            nc.vector.tensor_tensor(hacc[o], acc, hacc[o],
                                                    OP.add)
                        else:
                            h3t = silup.tile([128, B], BF16, tag="silu",
                                             name=f"h3_{o}")
                            nc.vector.tensor_tensor(h3t, acc, hacc[o],
                                                    OP.add)
                            h3.append(h3t)
                        # weave bases one quarter ahead: during B(l,q) build
                        # bases for quarter q+1 (slots freed by sweep q-1).
                        # At q3, batch next layer's silus (table-set
                        # grouping) and its quarter-0 bases.
                        if l + 1 < 3 and q == NQ - 1:
                            emit_silu(l + 1, o, hacc[o], bias=zb)
                            if o < KQ:
                                # na=0: relu/square only, so the silu-set
                                # stays loaded across this boundary window
                                emit_A(l + 1, o, hacc[o], 0)
                        if q < NQ - 1 and o < KQ:
                            na = 0 if (l == 0 and q == 0) else \
                                10 if l == 2 else 5
                            emit_A(l, KQ * (q + 1) + o,
                                   h_cur[KQ * (q + 1) + o], na)
                h_cur = hacc

            # ---- heads ----
            acch = psH.tile([128, B], F32, tag="acch")
            it2 = WIDTHS[-1] // 128
            for k in range(it2):
                nc.tensor.matmul(acch[0:2, :], wh_sb[:, 2 * k:2 * k + 2],
                                 h3[k], start=(k == 0), stop=(k == it2 - 1))
            res = consts.tile([2, B], F32, tag="res")
            nc.vector.tensor_scalar(res, acch[0:2, :], hb_sb[:, 0:1], None,
                                    OP.add)
            nc.sync.dma_start(out[:], res)

    return _patch_json(nc)


def _prep(inputs):
    """Host-side weight prep: fold scaler+lam, transpose, tile, bf16-cast."""
    f32 = np.float32
    bf16 = ml_dtypes.bfloat16
    feed = {}

    mw = np.asarray(inputs["mult_w"], f32).copy()  # [4096, 2048]
    mw[D:] *= LAM
    feed["wm"] = np.ascontiguousarray(
        mw.reshape(32, 128, IT, 128).transpose(0, 3, 2, 1)
        .reshape(32 * 128, D)).astype(bf16)
    mb = np.asarray(inputs["mult_b"], f32)
    feed["mbg"] = np.ascontiguousarray(mb[:D].reshape(16, 128).T).astype(f32)
    feed["mbv"] = np.ascontiguousarray(
        (LAM * mb[D:]).reshape(16, 128).T).astype(f32)

    for l, fo in enumerate(WIDTHS):
        sc_out = LAM if l < 2 else 1.0
        bw = np.asarray(inputs[f"base_w{l}"], f32) * sc_out
        sw = (np.asarray(inputs[f"spline_w{l}"], f32)
              * np.asarray(inputs[f"scaler{l}"], f32)[..., None] * sc_out)
        ot = fo // 128
        bwt = bw.reshape(ot, 128, IT, 128).transpose(0, 3, 2, 1)
        swt = sw.reshape(ot, 128, IT, 128, COEFF).transpose(0, 3, 2, 4, 1)
        arr = np.empty((ot, NQ, 128, SW), f32)
        arr[:, :, :, :512] = (bwt.reshape(ot, 128, NQ, KQ, 128)
                              .transpose(0, 2, 1, 3, 4)
                              .reshape(ot, NQ, 128, KQ * 128))
        arr[:, :, :, 512:] = (swt.reshape(ot, 128, NQ, KQ, COEFF, 128)
                              .transpose(0, 2, 1, 3, 4, 5)
                              .reshape(ot, NQ, 128, KQ * COEFF * 128))
        feed[f"ws{l}"] = np.ascontiguousarray(
            arr.reshape(ot * NQ * 128, SW)).astype(bf16)

    whh = np.stack([np.asarray(inputs["reg_w"], f32)[0],
                    np.asarray(inputs["aux_w"], f32)[0]], axis=1)  # [1024,2]
    feed["wh"] = np.ascontiguousarray(
        whh.reshape(8, 128, 2).transpose(1, 0, 2).reshape(128, 16)
    ).astype(bf16)
    feed["hb"] = np.array([[np.asarray(inputs["reg_b"], f32)[0]],
                           [np.asarray(inputs["aux_b"], f32)[0]]], f32)
    return feed


_NC = None


def kernel(**inputs):
    global _NC
    from concourse.bass_utils import run_bass_kernel_spmd

    if _NC is None:
        _NC = build()
    shared = _prep(inputs)
    x_full = np.asarray(inputs["x"], np.float32)
    per_core = []
    for c in range(N_CORES):
        m = dict(shared)
        m["xT"] = np.ascontiguousarray(
            x_full[c * B:(c + 1) * B].T).astype(ml_dtypes.bfloat16)
        per_core.append(m)
    res = run_bass_kernel_spmd(_NC, per_core, core_ids=list(range(N_CORES)))
    reg = np.concatenate([res.results[c]["out"][0] for c in range(N_CORES)])
    aux = np.concatenate([res.results[c]["out"][1] for c in range(N_CORES)])
    kernel.last_results = res
    return reg, aux


# revision 26
# speedup vs baseline: 1.2493x; 1.1014x over previous
"""BRD4KANModel Trainium2 kernel (v2).

Data-parallel over batch across 8 NeuronCores (512 rows each, weights
replicated). All weights are pre-transposed / pre-tiled / bf16-cast on the
host into matmul-ready lhsT layout ([in-feature partitions, out-feature
free]), with the spline scaler and the truncated-power scale lam folded in.
The device therefore runs ONLY real matmuls on the PE (no transposes), the
B-spline bases on ACT+DVE(+POOL), and PSUM evacuations.

B-spline bases via truncated powers: with h' = lam*h and c_m = lam*g_m,
z_m = relu(h' - c_m), the 6 cubic bases are the 4th forward differences
b_c = z³_c - 4z³_{c+1} + 6z³_{c+2} - 4z³_{c+3} + z³_{c+4}, computed as a
grouped 24-op DVE cascade per 128-feature tile. z² comes from one ACT
Square (bias = -c_m) and z³ = relu * z² runs on POOL (or DVE via env
BASS_CUBE=dve).

Layer matmuls are split into 4 k-quarters (i-tiles 0-3, 4-7, 8-11, 12-15)
so only ~2 quarters of bases tiles are ever live (SBUF), with an f32 SBUF
accumulator carrying partial sums between quarters. Bases for the next
consumer sweep are emitted one sweep ahead, overlapping DVE/ACT/POOL work
with PE matmuls.

This walrus build accepts only ONE semaphore wait per instruction, while
Tile's scheduler attaches several; _split_waits() post-processes the BIR
JSON, hoisting excess waits onto NoOps inserted just before each
instruction on the same engine.
"""

import json
import os

import numpy as np
import ml_dtypes

import concourse.bass as bass
import concourse.mybir as mybir
import concourse.tile as tile

F32 = mybir.dt.float32
BF16 = mybir.dt.bfloat16
AF = mybir.ActivationFunctionType
OP = mybir.AluOpType

N_CORES = 8
BATCH = 4096
B = BATCH // N_CORES  # 512 per core
D = 2048
WIDTHS = [2048, 2048, 1024]
COEFF = 6
GRID_SIZE = 3
SPLINE_ORDER = 3
HSTEP = 2.0 / GRID_SIZE
GRID = [m * HSTEP - 1.0 - SPLINE_ORDER * HSTEP
        for m in range(GRID_SIZE + 2 * SPLINE_ORDER + 1)]  # 10 knots, -3..3
LAM = float((6.0 * HSTEP ** 3) ** (-1.0 / 3.0))
NK = 10          # truncated-power knots
IT = 16          # 2048/128 input tiles per layer
NQ = 4           # k-quarters
KQ = IT // NQ    # i-tiles per quarter
SW = 512 + KQ * COEFF * 128  # combined strip width per (o,q): base + spline


def _split_waits(bir_bytes: bytes, keep: int = 1) -> bytes:
    d = json.loads(bir_bytes)
    for f in d["functions"]:
        for bb in f["blocks"]:
            new_insts = []
            for inst in bb["instructions"]:
                si = inst.get("sync_info")
                waits = (si or {}).get("on_wait") or []
                if len(waits) > keep:
                    extra = waits[:-keep]
                    inst["sync_info"]["on_wait"] = waits[-keep:]
                    for ci in range(0, len(extra), keep):
                        new_insts.append({
                            "name": f"{inst['name']}-w{ci}",
                            "opcode": "NoOp",
                            "engine": inst["engine"],
                            "ins": [],
                            "outs": [],
                            "debug": inst.get("debug"),
                            "sync_info": {"on_update": [],
                                          "on_wait": extra[ci:ci + keep]},
                        })
                new_insts.append(inst)
            bb["instructions"] = new_insts
    return json.dumps(d).encode()


def _patch_json(nc):
    orig = nc.to_json_bytes

    def patched():
        return _split_waits(orig())

    nc.to_json_bytes = patched
    return nc


def build():
    nc = bass.Bass()
    xT = nc.dram_tensor("xT", [D, B], BF16, kind="ExternalInput")
    wm = nc.dram_tensor("wm", [32 * 128, D], BF16, kind="ExternalInput")
    mbg = nc.dram_tensor("mbg", [128, 16], F32, kind="ExternalInput")
    mbv = nc.dram_tensor("mbv", [128, 16], F32, kind="ExternalInput")
    ws_d = []
    for l, fo in enumerate(WIDTHS):
        ot = fo // 128
        ws_d.append(nc.dram_tensor(f"ws{l}", [ot * NQ * 128, SW], BF16,
                                   kind="ExternalInput"))
    wh = nc.dram_tensor("wh", [128, 16], BF16, kind="ExternalInput")
    hb = nc.dram_tensor("hb", [2, 1], F32, kind="ExternalInput")
    out = nc.dram_tensor("out", [2, B], F32, kind="ExternalOutput")

    with tile.TileContext(nc) as tc:
        with tc.tile_pool(name="consts", bufs=1) as consts, \
             tc.tile_pool(name="wmp", bufs=2) as wmp, \
             tc.tile_pool(name="wsp", bufs=2) as wsp, \
             tc.tile_pool(name="hp", bufs=34) as hp, \
             tc.tile_pool(name="silup", bufs=21) as silup, \
             tc.tile_pool(name="basp", bufs=49) as basp, \
             tc.tile_pool(name="zp", bufs=10) as zp, \
             tc.tile_pool(name="rtp", bufs=6) as rtp, \
             tc.tile_pool(name="qp", bufs=6) as qp, \
             tc.tile_pool(name="psA", bufs=7, space="PSUM") as psA, \
             tc.tile_pool(name="psH", bufs=1, space="PSUM") as psH:

            # ---- constants ----
            cm = consts.tile([128, NK], F32, tag="cm")
            for m in range(NK):
                nc.vector.memset(cm[:, m:m + 1], float(-LAM * GRID[m]))
            mbg_sb = consts.tile([128, 16], F32, tag="mbg")
            nc.scalar.dma_start(mbg_sb, mbg[:])
            mbv_sb = consts.tile([128, 16], F32, tag="mbv")
            nc.scalar.dma_start(mbv_sb, mbv[:])
            wh_sb = consts.tile([128, 16], BF16, tag="wh")
            nc.scalar.dma_start(wh_sb, wh[:])
            hb_sb = consts.tile([2, 1], F32, tag="hb")
            nc.scalar.dma_start(hb_sb, hb[:])

            # ---- x^T tiles (host pre-transposed; share bas slots) ----
            xb = []
            for i in range(IT):
                t = basp.tile([128, B], BF16, tag="bas", name=f"x{i}")
                nc.scalar.dma_start(t, xT[i * 128:(i + 1) * 128, :])
                xb.append(t)

            silu_t = {}
            bas_t = {}

            def emit_silu(l, i, h_t, bias=0.0):
                # bias is always zero-valued; passing a gate tile written by
                # a late DVE op delays readiness so the list scheduler can't
                # pull Silu (its own ACT table-set) into an earlier window
                st = silup.tile([128, B], BF16, tag="silu",
                                name=f"silu{l}_{i}")
                nc.scalar.activation(st, h_t, AF.Silu, scale=1.0 / LAM,
                                     bias=bias)
                silu_t[(l, i)] = st

            z3_pend = {}

            def emit_A_z(l, i, h_t, na):
                """z^3 tiles for layer-l input tile i.

                na of the 10 cubes go through ACT (exp(3*ln(relu))), the
                rest use ACT relu+square plus a DVE multiply. Splitting
                balances the two engines (the GPSIMD SBUF port contention
                makes POOL offload a wash, so it gets nothing).
                """
                z3 = []
                for m in range(NK):
                    r = rtp.tile([128, B], F32, tag="rt", name=f"r{m}")
                    nc.scalar.activation(r, h_t, AF.Relu,
                                         bias=cm[:, m:m + 1])
                    z = zp.tile([128, B], F32, tag="z", name=f"z3_{m}")
                    if m < na:
                        lnr = rtp.tile([128, B], F32, tag="rt",
                                       name=f"lnr{m}")
                        nc.scalar.activation(lnr, r, AF.Ln)
                        nc.scalar.activation(z, lnr, AF.Exp, scale=3.0)
                    else:
                        t2 = rtp.tile([128, B], F32, tag="rt",
                                      name=f"t2{m}")
                        nc.scalar.activation(t2, h_t, AF.Square,
                                             bias=cm[:, m:m + 1])
                        nc.vector.tensor_tensor(z, r, t2, OP.mult)
                    z3.append(z)
                z3_pend[(l, i)] = z3
            def emit_A_casc(l, i):
                z3 = z3_pend.pop((l, i))
                for c in range(COEFF):
                    q = qp.tile([128, B], F32, tag="q", name=f"q{c}")
                    nc.vector.tensor_tensor(q, z3[c], z3[c + 4], OP.add)
                    r2 = qp.tile([128, B], F32, tag="q", name=f"r2{c}")
                    nc.vector.tensor_tensor(r2, z3[c + 1], z3[c + 3], OP.add)
                    nc.vector.scalar_tensor_tensor(q, r2, -4.0, q,
                                                   OP.mult, OP.add)
                    bt = basp.tile([128, B], BF16, tag="bas",
                                   name=f"bas{l}_{i}_{c}")
                    nc.vector.scalar_tensor_tensor(bt, z3[c + 2], 6.0, q,
                                                   OP.mult, OP.add)
                    bas_t[(l, i, c)] = bt

            def emit_A(l, i, h_t, na):
                emit_A_z(l, i, h_t, na)
                emit_A_casc(l, i)

            # ---- multiplicative layer ----
            h_cur = []
            for j in range(IT):
                wg = wmp.tile([128, D], BF16, tag="wm", name=f"wg{j}")
                nc.sync.dma_start(wg, wm[j * 128:(j + 1) * 128, :])
                accg = psA.tile([128, B], F32, tag="acc")
                for k in range(IT):
                    nc.tensor.matmul(accg, wg[:, k * 128:(k + 1) * 128],
                                     xb[k], start=(k == 0),
                                     stop=(k == IT - 1))
                sig = qp.tile([128, B], F32, tag="q", name=f"sig{j}")
                nc.scalar.activation(sig, accg, AF.Sigmoid,
                                     bias=mbg_sb[:, j:j + 1])
                wv = wmp.tile([128, D], BF16, tag="wm", name=f"wv{j}")
                nc.sync.dma_start(wv, wm[(16 + j) * 128:(17 + j) * 128, :])
                accv = psA.tile([128, B], F32, tag="acc")
                for k in range(IT):
                    nc.tensor.matmul(accv, wv[:, k * 128:(k + 1) * 128],
                                     xb[k], start=(k == 0),
                                     stop=(k == IT - 1))
                ht = hp.tile([128, B], F32, tag="h", name=f"h0_{j}")
                nc.vector.scalar_tensor_tensor(ht, accv, mbv_sb[:, j:j + 1],
                                               sig, OP.add, OP.mult)
                h_cur.append(ht)
                # A(0,q0) woven in at HALF-unit granularity: a full unit's
                # 34 DVE ops between consecutive h0 evac STTs starves the
                # PSUM pool and stalls the PE; half units keep the DVE lag
                # under the psA slack. na=0: no ln/exp while the sigmoid
                # table-set is live. Quarter 0 only: more would exhaust
                # bas slots (shared with x tiles).
                if 2 <= j < 2 + 2 * KQ:
                    u = (j - 2) // 2
                    if (j - 2) % 2 == 0:
                        emit_A_z(0, u, h_cur[u], 0)
                    else:
                        emit_A_casc(0, u)
            # zero tile written only after the last mult evac: gates the
            # silu batch out of the sigmoid-set window
            z00 = qp.tile([128, 1], F32, tag="zb", bufs=2)
            nc.vector.tensor_scalar(z00, h_cur[IT - 1][:, 0:1], 0.0, None,
                                    OP.mult)
            for j in range(IT):
                emit_silu(0, j, h_cur[j], bias=z00)

            # ---- KAN layers: 4-quarter k-split sweeps ----
            h3 = []
            for l in range(3):
                ot = WIDTHS[l] // 128
                hacc = [None] * ot
                for q in range(NQ):
                    zb = None
                    if l + 1 < 3 and q == NQ - 1:
                        # gate for next layer's silu batch: ready only after
                        # this layer's last bases tile (i.e. after all ln/exp
                        # ACT work for layer l is done)
                        zb = qp.tile([128, 1], F32, tag="zb", bufs=2,
                                     name=f"zb{l}")
                        nc.vector.tensor_scalar(
                            zb, bas_t[(l, IT - 1, COEFF - 1)][:, 0:1], 0.0,
                            None, OP.mult)
                    for o in range(ot):
                        strip = wsp.tile([128, SW], BF16, tag="ws",
                                         name=f"ws{l}_{q}_{o}")
                        row = (o * NQ + q) * 128
                        nc.sync.dma_start(strip, ws_d[l][row:row + 128, :])
                        acc = psA.tile([128, B], F32, tag="acc")
                        idx = 0
                        last = KQ * (1 + COEFF) - 1
                        for kk in range(KQ):
                            i = q * KQ + kk
                            nc.tensor.matmul(
                                acc, strip[:, kk * 128:(kk + 1) * 128],
                                silu_t[(l, i)], start=(idx == 0),
                                stop=(idx == last))
                            idx += 1
                            for c in range(COEFF):
                                o0 = 512 + (kk * COEFF + c) * 128
                                nc.tensor.matmul(
                                    acc, strip[:, o0:o0 + 128],
                                    bas_t[(l, i, c)], start=False,
                                    stop=(idx == last))
                                idx += 1
                        if q == 0:
                            hacc[o] = hp.tile([128, B], F32, tag="h",
                                              name=f"hacc{l}_{o}")
                            nc.scalar.copy(hacc[o], acc)
                        elif q < NQ - 1 or l < 2:
                            nc.vector.tensor_tensor(hacc[o], acc, hacc[o],
                                                    OP.add)
                        else:
                            h3t = silup.tile([128, B], BF16, tag="silu",
                                             name=f"h3_{o}")
                            nc.vector.tensor_tensor(h3t, acc, hacc[o],
                                                    OP.add)
                            h3.append(h3t)
                        # weave bases one quarter ahead: during B(l,q) build
                        # bases for quarter q+1 (slots freed by sweep q-1).
                        # At q3, batch next layer's silus (table-set
                        # grouping) and its quarter-0 bases.
                        if l + 1 < 3 and q == NQ - 1:
                            emit_silu(l + 1, o, hacc[o], bias=zb)
                            if o < KQ:
                                # na=7 balances DVE vs ACT in the boundary
                                # window (costs a couple of set switches)
                                emit_A(l + 1, o, hacc[o], 7)
                        if q < NQ - 1 and o < KQ:
                            na = 0 if (l == 0 and q == 0) else \
                                10 if l == 2 else 5
                            emit_A(l, KQ * (q + 1) + o,
                                   h_cur[KQ * (q + 1) + o], na)
                h_cur = hacc

            # ---- heads ----
            acch = psH.tile([128, B], F32, tag="acch")
            it2 = WIDTHS[-1] // 128
            for k in range(it2):
                nc.tensor.matmul(acch[0:2, :], wh_sb[:, 2 * k:2 * k + 2],
                                 h3[k], start=(k == 0), stop=(k == it2 - 1))
            res = consts.tile([2, B], F32, tag="res")
            nc.vector.tensor_scalar(res, acch[0:2, :], hb_sb[:, 0:1], None,
                                    OP.add)
            nc.sync.dma_start(out[:], res)

    return _patch_json(nc)


def _prep(inputs):
    """Host-side weight prep: fold scaler+lam, transpose, tile, bf16-cast."""
    f32 = np.float32
    bf16 = ml_dtypes.bfloat16
    feed = {}

    mw = np.asarray(inputs["mult_w"], f32).copy()  # [4096, 2048]
    mw[D:] *= LAM
    feed["wm"] = np.ascontiguousarray(
        mw.reshape(32, 128, IT, 128).transpose(0, 3, 2, 1)
        .reshape(32 * 128, D)).astype(bf16)
    mb = np.asarray(inputs["mult_b"], f32)
    feed["mbg"] = np.ascontiguousarray(mb[:D].reshape(16, 128).T).astype(f32)
    feed["mbv"] = np.ascontiguousarray(
        (LAM * mb[D:]).reshape(16, 128).T).astype(f32)

    for l, fo in enumerate(WIDTHS):
        sc_out = LAM if l < 2 else 1.0
        bw = np.asarray(inputs[f"base_w{l}"], f32) * sc_out
        sw = (np.asarray(inputs[f"spline_w{l}"], f32)
              * np.asarray(inputs[f"scaler{l}"], f32)[..., None] * sc_out)
        ot = fo // 128
        bwt = bw.reshape(ot, 128, IT, 128).transpose(0, 3, 2, 1)
        swt = sw.reshape(ot, 128, IT, 128, COEFF).transpose(0, 3, 2, 4, 1)
        arr = np.empty((ot, NQ, 128, SW), f32)
        arr[:, :, :, :512] = (bwt.reshape(ot, 128, NQ, KQ, 128)
                              .transpose(0, 2, 1, 3, 4)
                              .reshape(ot, NQ, 128, KQ * 128))
        arr[:, :, :, 512:] = (swt.reshape(ot, 128, NQ, KQ, COEFF, 128)
                              .transpose(0, 2, 1, 3, 4, 5)
                              .reshape(ot, NQ, 128, KQ * COEFF * 128))
        feed[f"ws{l}"] = np.ascontiguousarray(
            arr.reshape(ot * NQ * 128, SW)).astype(bf16)

    whh = np.stack([np.asarray(inputs["reg_w"], f32)[0],
                    np.asarray(inputs["aux_w"], f32)[0]], axis=1)  # [1024,2]
    feed["wh"] = np.ascontiguousarray(
        whh.reshape(8, 128, 2).transpose(1, 0, 2).reshape(128, 16)
    ).astype(bf16)
    feed["hb"] = np.array([[np.asarray(inputs["reg_b"], f32)[0]],
                           [np.asarray(inputs["aux_b"], f32)[0]]], f32)
    return feed


_NC = None


def kernel(**inputs):
    global _NC
    from concourse.bass_utils import run_bass_kernel_spmd

    if _NC is None:
        _NC = build()
    shared = _prep(inputs)
    x_full = np.asarray(inputs["x"], np.float32)
    per_core = []
    for c in range(N_CORES):
        m = dict(shared)
        m["xT"] = np.ascontiguousarray(
            x_full[c * B:(c + 1) * B].T).astype(ml_dtypes.bfloat16)
        per_core.append(m)
    res = run_bass_kernel_spmd(_NC, per_core, core_ids=list(range(N_CORES)))
    reg = np.concatenate([res.results[c]["out"][0] for c in range(N_CORES)])
    aux = np.concatenate([res.results[c]["out"][1] for c in range(N_CORES)])
    kernel.last_results = res
    return reg, aux


# revision 28
# speedup vs baseline: 1.2725x; 1.0185x over previous
"""BRD4KANModel Trainium2 kernel (v2).

Data-parallel over batch across 8 NeuronCores (512 rows each, weights
replicated). All weights are pre-transposed / pre-tiled / bf16-cast on the
host into matmul-ready lhsT layout ([in-feature partitions, out-feature
free]), with the spline scaler and the truncated-power scale lam folded in.
The device therefore runs ONLY real matmuls on the PE (no transposes), the
B-spline bases on ACT+DVE(+POOL), and PSUM evacuations.

B-spline bases via truncated powers: with h' = lam*h and c_m = lam*g_m,
z_m = relu(h' - c_m), the 6 cubic bases are the 4th forward differences
b_c = z³_c - 4z³_{c+1} + 6z³_{c+2} - 4z³_{c+3} + z³_{c+4}, computed as a
grouped 24-op DVE cascade per 128-feature tile. z² comes from one ACT
Square (bias = -c_m) and z³ = relu * z² runs on POOL (or DVE via env
BASS_CUBE=dve).

Layer matmuls are split into 4 k-quarters (i-tiles 0-3, 4-7, 8-11, 12-15)
so only ~2 quarters of bases tiles are ever live (SBUF), with an f32 SBUF
accumulator carrying partial sums between quarters. Bases for the next
consumer sweep are emitted one sweep ahead, overlapping DVE/ACT/POOL work
with PE matmuls.

This walrus build accepts only ONE semaphore wait per instruction, while
Tile's scheduler attaches several; _split_waits() post-processes the BIR
JSON, hoisting excess waits onto NoOps inserted just before each
instruction on the same engine.
"""

import json
import os

import numpy as np
import ml_dtypes

import concourse.bass as bass
import concourse.mybir as mybir
import concourse.tile as tile

F32 = mybir.dt.float32
BF16 = mybir.dt.bfloat16
AF = mybir.ActivationFunctionType
OP = mybir.AluOpType

N_CORES = 8
BATCH = 4096
B = BATCH // N_CORES  # 512 per core
D = 2048
WIDTHS = [2048, 2048, 1024]
COEFF = 6
GRID_SIZE = 3
SPLINE_ORDER = 3
HSTEP = 2.0 / GRID_SIZE
GRID = [m * HSTEP - 1.0 - SPLINE_ORDER * HSTEP
        for m in range(GRID_SIZE + 2 * SPLINE_ORDER + 1)]  # 10 knots, -3..3
LAM = float((6.0 * HSTEP ** 3) ** (-1.0 / 3.0))
NK = 10          # truncated-power knots
IT = 16          # 2048/128 input tiles per layer
NQ = 4           # k-quarters
KQ = IT // NQ    # i-tiles per quarter
SW = 512 + KQ * COEFF * 128  # combined strip width per (o,q): base + spline


def _split_waits(bir_bytes: bytes, keep: int = 1) -> bytes:
    d = json.loads(bir_bytes)
    for f in d["functions"]:
        for bb in f["blocks"]:
            new_insts = []
            for inst in bb["instructions"]:
                si = inst.get("sync_info")
                waits = (si or {}).get("on_wait") or []
                if len(waits) > keep:
                    extra = waits[:-keep]
                    inst["sync_info"]["on_wait"] = waits[-keep:]
                    for ci in range(0, len(extra), keep):
                        new_insts.append({
                            "name": f"{inst['name']}-w{ci}",
                            "opcode": "NoOp",
                            "engine": inst["engine"],
                            "ins": [],
                            "outs": [],
                            "debug": inst.get("debug"),
                            "sync_info": {"on_update": [],
                                          "on_wait": extra[ci:ci + keep]},
                        })
                new_insts.append(inst)
            bb["instructions"] = new_insts
    return json.dumps(d).encode()


def _patch_json(nc):
    orig = nc.to_json_bytes

    def patched():
        return _split_waits(orig())

    nc.to_json_bytes = patched
    return nc


def build():
    nc = bass.Bass()
    xT = nc.dram_tensor("xT", [D, B], BF16, kind="ExternalInput")
    wm = nc.dram_tensor("wm", [32 * 128, D], BF16, kind="ExternalInput")
    mbg = nc.dram_tensor("mbg", [128, 16], F32, kind="ExternalInput")
    mbv = nc.dram_tensor("mbv", [128, 16], F32, kind="ExternalInput")
    ws_d = []
    for l, fo in enumerate(WIDTHS):
        ot = fo // 128
        ws_d.append(nc.dram_tensor(f"ws{l}", [ot * NQ * 128, SW], BF16,
                                   kind="ExternalInput"))
    wh = nc.dram_tensor("wh", [128, 16], BF16, kind="ExternalInput")
    hb = nc.dram_tensor("hb", [2, 1], F32, kind="ExternalInput")
    out = nc.dram_tensor("out", [2, B], F32, kind="ExternalOutput")

    with tile.TileContext(nc) as tc:
        with tc.tile_pool(name="consts", bufs=1) as consts, \
             tc.tile_pool(name="wmp", bufs=2) as wmp, \
             tc.tile_pool(name="wsp", bufs=3) as wsp, \
             tc.tile_pool(name="hp", bufs=33) as hp, \
             tc.tile_pool(name="silup", bufs=19) as silup, \
             tc.tile_pool(name="basp", bufs=49) as basp, \
             tc.tile_pool(name="zp", bufs=10) as zp, \
             tc.tile_pool(name="rtp", bufs=5) as rtp, \
             tc.tile_pool(name="qp", bufs=6) as qp, \
             tc.tile_pool(name="psA", bufs=7, space="PSUM") as psA, \
             tc.tile_pool(name="psH", bufs=1, space="PSUM") as psH:

            # ---- constants ----
            cm = consts.tile([128, NK], F32, tag="cm")
            for m in range(NK):
                nc.vector.memset(cm[:, m:m + 1], float(-LAM * GRID[m]))
            mbg_sb = consts.tile([128, 16], F32, tag="mbg")
            nc.scalar.dma_start(mbg_sb, mbg[:])
            mbv_sb = consts.tile([128, 16], F32, tag="mbv")
            nc.scalar.dma_start(mbv_sb, mbv[:])
            wh_sb = consts.tile([128, 16], BF16, tag="wh")
            nc.scalar.dma_start(wh_sb, wh[:])
            hb_sb = consts.tile([2, 1], F32, tag="hb")
            nc.scalar.dma_start(hb_sb, hb[:])

            # ---- x^T tiles (host pre-transposed; share bas slots) ----
            xb = []
            for i in range(IT):
                t = basp.tile([128, B], BF16, tag="bas", name=f"x{i}")
                nc.scalar.dma_start(t, xT[i * 128:(i + 1) * 128, :])
                xb.append(t)

            silu_t = {}
            bas_t = {}

            def emit_silu(l, i, h_t, bias=0.0):
                # bias is always zero-valued; passing a gate tile written by
                # a late DVE op delays readiness so the list scheduler can't
                # pull Silu (its own ACT table-set) into an earlier window
                st = silup.tile([128, B], BF16, tag="silu",
                                name=f"silu{l}_{i}")
                nc.scalar.activation(st, h_t, AF.Silu, scale=1.0 / LAM,
                                     bias=bias)
                silu_t[(l, i)] = st

            z3_pend = {}

            def emit_A_z(l, i, h_t, na):
                """z^3 tiles for layer-l input tile i.

                na of the 10 cubes go through ACT (exp(3*ln(relu))), the
                rest use ACT relu+square plus a DVE multiply. Splitting
                balances the two engines (the GPSIMD SBUF port contention
                makes POOL offload a wash, so it gets nothing).
                """
                z3 = []
                for m in range(NK):
                    r = rtp.tile([128, B], F32, tag="rt", name=f"r{m}")
                    nc.scalar.activation(r, h_t, AF.Relu,
                                         bias=cm[:, m:m + 1])
                    z = zp.tile([128, B], F32, tag="z", name=f"z3_{m}")
                    if m < na:
                        lnr = rtp.tile([128, B], F32, tag="rt",
                                       name=f"lnr{m}")
                        nc.scalar.activation(lnr, r, AF.Ln)
                        nc.scalar.activation(z, lnr, AF.Exp, scale=3.0)
                    else:
                        t2 = rtp.tile([128, B], F32, tag="rt",
                                      name=f"t2{m}")
                        nc.scalar.activation(t2, h_t, AF.Square,
                                             bias=cm[:, m:m + 1])
                        nc.vector.tensor_tensor(z, r, t2, OP.mult)
                    z3.append(z)
                z3_pend[(l, i)] = z3
            def emit_A_casc(l, i):
                z3 = z3_pend.pop((l, i))
                for c in range(COEFF):
                    q = qp.tile([128, B], F32, tag="q", name=f"q{c}")
                    nc.vector.tensor_tensor(q, z3[c], z3[c + 4], OP.add)
                    r2 = qp.tile([128, B], F32, tag="q", name=f"r2{c}")
                    nc.vector.tensor_tensor(r2, z3[c + 1], z3[c + 3], OP.add)
                    nc.vector.scalar_tensor_tensor(q, r2, -4.0, q,
                                                   OP.mult, OP.add)
                    bt = basp.tile([128, B], BF16, tag="bas",
                                   name=f"bas{l}_{i}_{c}")
                    nc.vector.scalar_tensor_tensor(bt, z3[c + 2], 6.0, q,
                                                   OP.mult, OP.add)
                    bas_t[(l, i, c)] = bt

            def emit_A(l, i, h_t, na):
                emit_A_z(l, i, h_t, na)
                emit_A_casc(l, i)

            # ---- multiplicative layer ----
            h_cur = []
            for j in range(IT):
                wg = wmp.tile([128, D], BF16, tag="wm", name=f"wg{j}")
                nc.sync.dma_start(wg, wm[j * 128:(j + 1) * 128, :])
                accg = psA.tile([128, B], F32, tag="acc")
                for k in range(IT):
                    nc.tensor.matmul(accg, wg[:, k * 128:(k + 1) * 128],
                                     xb[k], start=(k == 0),
                                     stop=(k == IT - 1))
                sig = qp.tile([128, B], F32, tag="q", name=f"sig{j}")
                nc.scalar.activation(sig, accg, AF.Sigmoid,
                                     bias=mbg_sb[:, j:j + 1])
                wv = wmp.tile([128, D], BF16, tag="wm", name=f"wv{j}")
                nc.sync.dma_start(wv, wm[(16 + j) * 128:(17 + j) * 128, :])
                accv = psA.tile([128, B], F32, tag="acc")
                for k in range(IT):
                    nc.tensor.matmul(accv, wv[:, k * 128:(k + 1) * 128],
                                     xb[k], start=(k == 0),
                                     stop=(k == IT - 1))
                ht = hp.tile([128, B], F32, tag="h", name=f"h0_{j}")
                nc.vector.scalar_tensor_tensor(ht, accv, mbv_sb[:, j:j + 1],
                                               sig, OP.add, OP.mult)
                h_cur.append(ht)
                # A(0,q0) woven in at HALF-unit granularity: a full unit's
                # 34 DVE ops between consecutive h0 evac STTs starves the
                # PSUM pool and stalls the PE; half units keep the DVE lag
                # under the psA slack. na=0: no ln/exp while the sigmoid
                # table-set is live. Quarter 0 only: more would exhaust
                # bas slots (shared with x tiles).
                if 2 <= j < 2 + 2 * KQ:
                    u = (j - 2) // 2
                    if (j - 2) % 2 == 0:
                        emit_A_z(0, u, h_cur[u], 0)
                    else:
                        emit_A_casc(0, u)
            # zero tile written only after the last mult evac: gates the
            # silu batch out of the sigmoid-set window
            z00 = qp.tile([128, 1], F32, tag="zb", bufs=2)
            nc.vector.tensor_scalar(z00, h_cur[IT - 1][:, 0:1], 0.0, None,
                                    OP.mult)
            for j in range(IT):
                emit_silu(0, j, h_cur[j], bias=z00)

            # ---- KAN layers: 4-quarter k-split sweeps ----
            h3 = []
            for l in range(3):
                ot = WIDTHS[l] // 128
                hacc = [None] * ot
                for q in range(NQ):
                    zb = None
                    if l + 1 < 3 and q == NQ - 1:
                        # gate for next layer's silu batch: ready only after
                        # this layer's last bases tile (i.e. after all ln/exp
                        # ACT work for layer l is done)
                        zb = qp.tile([128, 1], F32, tag="zb", bufs=2,
                                     name=f"zb{l}")
                        nc.vector.tensor_scalar(
                            zb, bas_t[(l, IT - 1, COEFF - 1)][:, 0:1], 0.0,
                            None, OP.mult)
                    for o in range(ot):
                        strip = wsp.tile([128, SW], BF16, tag="ws",
                                         name=f"ws{l}_{q}_{o}")
                        row = (o * NQ + q) * 128
                        nc.sync.dma_start(strip, ws_d[l][row:row + 128, :])
                        acc = psA.tile([128, B], F32, tag="acc")
                        idx = 0
                        last = KQ * (1 + COEFF) - 1
                        for kk in range(KQ):
                            i = q * KQ + kk
                            nc.tensor.matmul(
                                acc, strip[:, kk * 128:(kk + 1) * 128],
                                silu_t[(l, i)], start=(idx == 0),
                                stop=(idx == last))
                            idx += 1
                            for c in range(COEFF):
                                o0 = 512 + (kk * COEFF + c) * 128
                                nc.tensor.matmul(
                                    acc, strip[:, o0:o0 + 128],
                                    bas_t[(l, i, c)], start=False,
                                    stop=(idx == last))
                                idx += 1
                        if q == 0:
                            hacc[o] = hp.tile([128, B], F32, tag="h",
                                              name=f"hacc{l}_{o}")
                            nc.scalar.copy(hacc[o], acc)
                        elif q < NQ - 1 or l < 2:
                            nc.vector.tensor_tensor(hacc[o], acc, hacc[o],
                                                    OP.add)
                        else:
                            h3t = silup.tile([128, B], BF16, tag="silu",
                                             name=f"h3_{o}")
                            nc.vector.tensor_tensor(h3t, acc, hacc[o],
                                                    OP.add)
                            h3.append(h3t)
                        # weave bases one quarter ahead: during B(l,q) build
                        # bases for quarter q+1 (slots freed by sweep q-1).
                        # At q3, batch next layer's silus (table-set
                        # grouping) and its quarter-0 bases.
                        if l + 1 < 3 and q == NQ - 1:
                            emit_silu(l + 1, o, hacc[o], bias=zb)
                            if o < KQ:
                                # na=7 balances DVE vs ACT in the boundary
                                # window (costs a couple of set switches)
                                emit_A(l + 1, o, hacc[o], 7)
                        if q < NQ - 1 and o < KQ:
                            na = 0 if (l == 0 and q == 0) else \
                                8 if l == 2 else 5
                            emit_A(l, KQ * (q + 1) + o,
                                   h_cur[KQ * (q + 1) + o], na)
                h_cur = hacc

            # ---- heads ----
            acch = psH.tile([128, B], F32, tag="acch")
            it2 = WIDTHS[-1] // 128
            for k in range(it2):
                nc.tensor.matmul(acch[0:2, :], wh_sb[:, 2 * k:2 * k + 2],
                                 h3[k], start=(k == 0), stop=(k == it2 - 1))
            res = consts.tile([2, B], F32, tag="res")
            nc.vector.tensor_scalar(res, acch[0:2, :], hb_sb[:, 0:1], None,
                                    OP.add)
            nc.sync.dma_start(out[:], res)

    return _patch_json(nc)


def _prep(inputs):
    """Host-side weight prep: fold scaler+lam, transpose, tile, bf16-cast."""
    f32 = np.float32
    bf16 = ml_dtypes.bfloat16
    feed = {}

    mw = np.asarray(inputs["mult_w"], f32).copy()  # [4096, 2048]
    mw[D:] *= LAM
    feed["wm"] = np.ascontiguousarray(
        mw.reshape(32, 128, IT, 128).transpose(0, 3, 2, 1)
        .reshape(32 * 128, D)).astype(bf16)
    mb = np.asarray(inputs["mult_b"], f32)
    feed["mbg"] = np.ascontiguousarray(mb[:D].reshape(16, 128).T).astype(f32)
    feed["mbv"] = np.ascontiguousarray(
        (LAM * mb[D:]).reshape(16, 128).T).astype(f32)

    for l, fo in enumerate(WIDTHS):
        sc_out = LAM if l < 2 else 1.0
        bw = np.asarray(inputs[f"base_w{l}"], f32) * sc_out
        sw = (np.asarray(inputs[f"spline_w{l}"], f32)
              * np.asarray(inputs[f"scaler{l}"], f32)[..., None] * sc_out)
        ot = fo // 128
        bwt = bw.reshape(ot, 128, IT, 128).transpose(0, 3, 2, 1)
        swt = sw.reshape(ot, 128, IT, 128, COEFF).transpose(0, 3, 2, 4, 1)
        arr = np.empty((ot, NQ, 128, SW), f32)
        arr[:, :, :, :512] = (bwt.reshape(ot, 128, NQ, KQ, 128)
                              .transpose(0, 2, 1, 3, 4)
                              .reshape(ot, NQ, 128, KQ * 128))
        arr[:, :, :, 512:] = (swt.reshape(ot, 128, NQ, KQ, COEFF, 128)
                              .transpose(0, 2, 1, 3, 4, 5)
                              .reshape(ot, NQ, 128, KQ * COEFF * 128))
        feed[f"ws{l}"] = np.ascontiguousarray(
            arr.reshape(ot * NQ * 128, SW)).astype(bf16)

    whh = np.stack([np.asarray(inputs["reg_w"], f32)[0],
                    np.asarray(inputs["aux_w"], f32)[0]], axis=1)  # [1024,2]
    feed["wh"] = np.ascontiguousarray(
        whh.reshape(8, 128, 2).transpose(1, 0, 2).reshape(128, 16)
    ).astype(bf16)
    feed["hb"] = np.array([[np.asarray(inputs["reg_b"], f32)[0]],
                           [np.asarray(inputs["aux_b"], f32)[0]]], f32)
    return feed


_NC = None


def kernel(**inputs):
    global _NC
    from concourse.bass_utils import run_bass_kernel_spmd

    if _NC is None:
        _NC = build()
    shared = _prep(inputs)
    x_full = np.asarray(inputs["x"], np.float32)
    per_core = []
    for c in range(N_CORES):
        m = dict(shared)
        m["xT"] = np.ascontiguousarray(
            x_full[c * B:(c + 1) * B].T).astype(ml_dtypes.bfloat16)
        per_core.append(m)
    res = run_bass_kernel_spmd(_NC, per_core, core_ids=list(range(N_CORES)))
    reg = np.concatenate([res.results[c]["out"][0] for c in range(N_CORES)])
    aux = np.concatenate([res.results[c]["out"][1] for c in range(N_CORES)])
    kernel.last_results = res
    return reg, aux
